# revision 1
# baseline (speedup 1.0000x reference)
import os
import sys

for _p in ("/opt/trn_rl_repo", os.path.expanduser("~/.axon_site/_ro/trn_rl_repo")):
    if os.path.isdir(_p) and _p not in sys.path:
        sys.path.insert(0, _p)

import numpy as np
import ml_dtypes

import concourse.bass as bass
from concourse import bacc
import concourse.tile as tile
import concourse.mybir as mybir
from concourse.bass_utils import run_bass_kernel_spmd

# Problem shape (hardcoded per contract)
B, T, D, H, DK = 4, 2048, 1024, 16, 64
NCORES = 8

# Sharding: core = (batch b, head-group hg). Each core handles 8 heads of one
# batch over the full sequence, row-shards W_o, and the host sums the two
# partial outputs per batch (the "all-reduce" of the tensor-parallel scheme).
HC = H // 2       # 8 heads per core
DC = HC * DK      # 512 hidden dims per core

P = 128
NDT = D // P      # 8 din tiles
NHT = DC // P     # 4 dout tiles for this core's heads
NKT = T // P      # 16 key-token tiles
NPAIR = HC // 2   # 4 head pairs (pair p <-> dout tile p)
QCH = 512         # free-dim chunk per matmul
NQC = T // QCH    # 4 q-chunks (all tokens are queries now)
NTT = T // P      # 16 output token tiles

bf16 = mybir.dt.bfloat16
f32 = mybir.dt.float32
FT = mybir.ActivationFunctionType
ADD = mybir.AluOpType.add
MUL = mybir.AluOpType.mult

_CACHE = {}


def build_kernel():
    nc = bacc.Bacc("TRN2", target_bir_lowering=False, debug=False, num_devices=1)

    # Per-core inputs (already sliced to this core's heads where applicable)
    xT = nc.dram_tensor("xT", [D, T], bf16, kind="ExternalInput")  # x[b].T
    # weights arrive pre-tiled from the host so every DMA is contiguous:
    # Wq/Wk: [dt, p, a, m] = W[a*128+p, dt*128+m]; Wv: [p, a, m] = W[a*128+p, m]
    # Wo: [ch, p, a, m] = Wo_shard[a*128+p, ch*512+m]
    Wq = nc.dram_tensor("Wq", [NHT, P, NDT, P], bf16, kind="ExternalInput")
    Wk = nc.dram_tensor("Wk", [NHT, P, NDT, P], bf16, kind="ExternalInput")
    Wv = nc.dram_tensor("Wv", [P, NDT, DC], bf16, kind="ExternalInput")
    Wo = nc.dram_tensor("Wo", [2, P, NHT, QCH], bf16, kind="ExternalInput")
    # bq/bk pre-striped on host to [128, NHT] (col t = bias[t*128:(t+1)*128])
    bqp = nc.dram_tensor("bqp", [P, NHT], f32, kind="ExternalInput")
    bkp = nc.dram_tensor("bkp", [P, NHT], f32, kind="ExternalInput")
    bv = nc.dram_tensor("bv", [1, DC], f32, kind="ExternalInput")
    bo = nc.dram_tensor("bo", [1, D], f32, kind="ExternalInput")  # pre-halved
    out = nc.dram_tensor("out", [T, D], mybir.dt.float16, kind="ExternalOutput")

    with tile.TileContext(nc) as tc:
        with (
            tc.tile_pool(name="big", bufs=1) as big,
            tc.tile_pool(name="tmp", bufs=3) as tmp,
            tc.tile_pool(name="res", bufs=4) as resp,
            tc.tile_pool(name="dram", bufs=1, space="DRAM") as dramp,
            tc.tile_pool(name="acc", bufs=2, space="PSUM") as accp,
            tc.tile_pool(name="sg", bufs=2, space="PSUM") as sgp,
            tc.tile_pool(name="ops", bufs=1, space="PSUM") as opsp,
        ):
            # K-projection weights prefetched first (first matmuls need them),
            # then x^T tiles quarter-major so the earliest columns land first.
            wk_w = big.tile([P, NHT, NDT, P], bf16, name="wk_w")
            for dt in range(NHT):
                nc.sync.dma_start(wk_w[:, dt], Wk[dt])
            xt_sb = [big.tile([P, T], bf16, name=f"xt{i}") for i in range(NDT)]
            wq_w = big.tile([P, NHT, NDT, P], bf16, name="wq_w")
            for q in range(4):
                sl = slice(q * QCH, (q + 1) * QCH)
                for i in range(NDT):
                    eng = nc.sync if i % 2 == 0 else nc.gpsimd
                    eng.dma_start(xt_sb[i][:, sl], xT[i * P : (i + 1) * P, sl])
            # Q weights after x but ahead of the larger deferred loads
            for dt in range(NHT):
                nc.gpsimd.dma_start(wq_w[:, dt], Wq[dt])

            bq_sb = big.tile([P, NHT], f32, name="bq_sb")
            bk_sb = big.tile([P, NHT], f32, name="bk_sb")
            nc.sync.dma_start(bq_sb[:], bqp[:])
            nc.sync.dma_start(bk_sb[:], bkp[:])
            bv_rep = big.tile([P, DC], f32, name="bv_rep")
            bo_rep = big.tile([P, D], f32, name="bo_rep")
            wv_ch = big.tile([P, NDT, DC], bf16, name="wv_ch")
            wo_ch = [big.tile([P, NHT, QCH], bf16, name=f"wo{ch}") for ch in range(2)]

            # persistent activations
            kt_sb = [big.tile([P, T], bf16, name=f"kt{p}") for p in range(NPAIR)]
            qt_sb = [big.tile([P, T], bf16, name=f"qt{p}") for p in range(NPAIR)]
            vp_sb = [big.tile([P, HC, DK + 1], bf16, name=f"vp{t}") for t in range(NKT)]
            for t in range(NKT):
                nc.any.memset(vp_sb[t][:], 1.0)
            ob_sb = [
                [big.tile([P, QCH], bf16, name=f"ob{p}_{c}") for c in range(NQC)]
                for p in range(NPAIR)
            ]
            den_sb = [big.tile([HC, QCH], f32, name=f"den{c}") for c in range(NQC)]
            rec_dr = [dramp.tile([HC, QCH], f32, name=f"recd{c}") for c in range(NQC)]

            # ---------- phase 1: projections ----------
            def proj_tile(w_t, bias_sb, dst_tiles, dt, ch):
                ps = accp.tile([P, QCH], f32, name="proj_ps")
                for di in range(NDT):
                    nc.tensor.matmul(
                        ps[:],
                        w_t[:, di, :],
                        xt_sb[di][:, ch * QCH : (ch + 1) * QCH],
                        start=(di == 0),
                        stop=(di == NDT - 1),
                    )
                nc.vector.tensor_tensor(
                    dst_tiles[dt][:, ch * QCH : (ch + 1) * QCH],
                    ps[:],
                    bias_sb[:, dt : dt + 1].to_broadcast((P, QCH)),
                    ADD,
                )

            # K projection chunk-major: all dout tiles of x-quarter q before
            # quarter q+1 is needed (wk_w is fully prefetched)
            for ch in range(NQC):
                for dt in range(NHT):
                    proj_tile(wk_w[:, dt], bk_sb, kt_sb, dt, ch)
            # staged loads deferred so the first projections' DMAs go first
            nc.sync.dma_start(wv_ch[:], Wv[:])
            nc.sync.dma_start(bv_rep[:], bv[:].to_broadcast((P, DC)))
            nc.sync.dma_start(bo_rep[:], bo[:].to_broadcast((P, D)))
            for ch in range(2):
                nc.sync.dma_start(wo_ch[ch][:], Wo[ch])
            # V in natural layout, scattered into the padded V' tiles
            # (before Q: the attention-output matmuls need all V tiles, while
            # scores only need the Q tile of their own pair)
            for tt in range(NKT):
                ps = accp.tile([P, QCH], f32, name="proj_ps")
                for di in range(NDT):
                    nc.tensor.matmul(
                        ps[:],
                        xt_sb[di][:, tt * P : (tt + 1) * P],
                        wv_ch[:, di, :],
                        start=(di == 0),
                        stop=(di == NDT - 1),
                    )
                # all heads laid out as [V(64) | 1]
                nc.vector.tensor_tensor(
                    vp_sb[tt][:, :, 0:DK],
                    ps[:].rearrange("p (h d) -> p h d", d=DK),
                    bv_rep[:].rearrange("p (h d) -> p h d", d=DK),
                    ADD,
                )

            for ch in range(NQC):
                for dt in range(NHT):
                    proj_tile(wq_w[:, dt], bq_sb, qt_sb, dt, ch)

            # ---------- phase 2: attention ----------
            for c in range(NQC):
                qsl = slice(c * QCH, (c + 1) * QCH)
                for p in range(NPAIR):
                    hA, hB = 2 * p, 2 * p + 1
                    oA = opsp.tile([P, QCH], f32, name="oA")
                    oB = opsp.tile([P, QCH], f32, name="oB")
                    for g in range(NKT // 2):
                        sgA = sgp.tile([P, 2, QCH], f32, tag="sg")
                        sgB = sgp.tile([P, 2, QCH], f32, tag="sg")
                        for j in range(2):
                            kt = 2 * g + j
                            ksl = slice(kt * P, (kt + 1) * P)
                            nc.tensor.matmul(
                                sgA[:, j, :],
                                kt_sb[p][0:DK, ksl],
                                qt_sb[p][0:DK, qsl],
                                start=True,
                                stop=True,
                                tile_position=(0, 0),
                            )
                            nc.tensor.matmul(
                                sgB[:, j, :],
                                kt_sb[p][DK:P, ksl],
                                qt_sb[p][DK:P, qsl],
                                start=True,
                                stop=True,
                                tile_position=(64, 0),
                            )
                        ptA = tmp.tile([P, 2, QCH], bf16, tag="pt")
                        ptB = tmp.tile([P, 2, QCH], bf16, tag="pt")
                        nc.scalar.activation(ptA[:], sgA[:], FT.Exp, scale=0.125)
                        nc.scalar.activation(ptB[:], sgB[:], FT.Exp, scale=0.125)
                        for j in range(2):
                            kt = 2 * g + j
                            nc.tensor.matmul(
                                oA[0:65, :],
                                vp_sb[kt][:, hA, :],
                                ptA[:, j, :],
                                start=(kt == 0),
                                stop=(kt == NKT - 1),
                            )
                            nc.tensor.matmul(
                                oB[0:65, :],
                                vp_sb[kt][:, hB, :],
                                ptB[:, j, :],
                                start=(kt == 0),
                                stop=(kt == NKT - 1),
                            )
                    # raw O^T to SBUF (bf16); head B via staging + shift DMA.
                    # Denominators (row 64) stage through fp32 row tiles.
                    nc.vector.tensor_copy(ob_sb[p][c][0:DK, :], oA[0:DK, :])
                    stgB = tmp.tile([DK, QCH], bf16, tag="bstg")
                    nc.vector.tensor_copy(stgB[:], oB[0:DK, :])
                    nc.gpsimd.dma_start(ob_sb[p][c][DK:P, :], stgB[:])
                    stgDA = tmp.tile([65, QCH], f32, tag="dstgA")
                    stgDB = tmp.tile([65, QCH], f32, tag="dstgB")
                    nc.vector.tensor_copy(stgDA[64:65, :], oA[64:65, :])
                    nc.vector.tensor_copy(stgDB[64:65, :], oB[64:65, :])
                    nc.gpsimd.dma_start(den_sb[c][hA : hA + 1, :], stgDA[64:65, :])
                    nc.gpsimd.dma_start(den_sb[c][hB : hB + 1, :], stgDB[64:65, :])

                # normalize: reciprocal (in place), DRAM-bounced broadcast
                nc.vector.reciprocal(den_sb[c][:], den_sb[c][:])
                nc.gpsimd.dma_start(rec_dr[c][:], den_sb[c][:])
                for p in range(NPAIR):
                    hA, hB = 2 * p, 2 * p + 1
                    rep = tmp.tile([P, QCH], f32, tag="rep")
                    nc.gpsimd.dma_start(
                        rep[0:DK, :], rec_dr[c][hA : hA + 1, :].to_broadcast((DK, QCH))
                    )
                    nc.gpsimd.dma_start(
                        rep[DK:P, :], rec_dr[c][hB : hB + 1, :].to_broadcast((DK, QCH))
                    )
                    nc.vector.tensor_tensor(
                        ob_sb[p][c][:], ob_sb[p][c][:], rep[:], MUL
                    )

            # ---------- phase 3: output projection (row-sharded W_o) ----------
            # Emit chunk-3 token tiles last: their ob tiles normalize at the
            # very end of phase 2, and the scheduler follows emission order.
            out_v = out[:].rearrange("(tt p) d -> p tt d", p=P)

            def out_group(ch, tg):
                # 2 token tiles -> one grouped result tile -> one DMA
                res = resp.tile([P, 2, QCH], mybir.dt.float16, tag="ores")
                for k in range(2):
                    ttk = 2 * tg + k
                    c, s = ttk // 4, (ttk % 4) * P
                    ps = accp.tile([P, QCH], f32, name="proj_ps")
                    for p in range(NPAIR):
                        nc.tensor.matmul(
                            ps[:],
                            ob_sb[p][c][:, s : s + P],
                            wo_ch[ch][:, p, :],
                            start=(p == 0),
                            stop=(p == NPAIR - 1),
                        )
                    nc.vector.tensor_tensor(
                        res[:, k, :], ps[:], bo_rep[:, ch * QCH : (ch + 1) * QCH], ADD
                    )
                nc.sync.dma_start(
                    out_v[:, 2 * tg : 2 * tg + 2, ch * QCH : (ch + 1) * QCH],
                    res[:],
                )

            for ch in range(2):
                for tg in range(6):
                    out_group(ch, tg)
            for ch in range(2):
                for tg in range(6, 8):
                    out_group(ch, tg)

    nc.compile()
    return nc


def _prep_inputs(x, Wq, bq, Wk, bk, Wv, bv, Wo, bo):
    """Shard + lay out inputs for the 8 cores (batch x head-group)."""
    x = np.asarray(x, dtype=np.float32)
    to_bf = lambda a: np.ascontiguousarray(a).astype(ml_dtypes.bfloat16)
    Wq, Wk, Wv, Wo = (np.asarray(w, np.float32) for w in (Wq, Wk, Wv, Wo))
    bq, bk, bv, bo = (np.asarray(v, np.float32) for v in (bq, bk, bv, bo))
    bo_half = np.ascontiguousarray((bo * 0.5).reshape(1, D))
    xTb = [to_bf(x[b].T) for b in range(B)]
    in_maps = []
    for core in range(NCORES):
        b, hg = core // 2, core % 2
        csl = slice(hg * DC, (hg + 1) * DC)

        def tile_qk(W):
            # [D, DC] -> [dt, p, a, m]
            return to_bf(
                W[:, csl].reshape(NDT, P, NHT, P).transpose(2, 1, 0, 3)
            )

        in_maps.append(
            {
                "xT": xTb[b],
                "Wq": tile_qk(Wq),
                "Wk": tile_qk(Wk),
                "Wv": to_bf(Wv[:, csl].reshape(NDT, P, DC).transpose(1, 0, 2)),
                "Wo": to_bf(
                    Wo[csl, :].reshape(NHT, P, 2, QCH).transpose(2, 1, 0, 3)
                ),
                "bqp": np.ascontiguousarray(bq[csl].reshape(NHT, P).T),
                "bkp": np.ascontiguousarray(bk[csl].reshape(NHT, P).T),
                "bv": np.ascontiguousarray(bv[csl].reshape(1, DC)),
                "bo": bo_half,
            }
        )
    return in_maps


def kernel(x, Wq, bq, Wk, bk, Wv, bv, Wo, bo):
    if "nc" not in _CACHE:
        _CACHE["nc"] = build_kernel()
    nc = _CACHE["nc"]
    in_maps = _prep_inputs(x, Wq, bq, Wk, bk, Wv, bv, Wo, bo)
    res = run_bass_kernel_spmd(nc, in_maps, list(range(NCORES)))
    out = np.empty((B, T, D), dtype=np.float32)
    for b in range(B):
        out[b] = res.results[2 * b]["out"].astype(np.float32) + res.results[
            2 * b + 1
        ]["out"].astype(np.float32)
    return out



# revision 6
# speedup vs baseline: 1.3861x; 1.3861x over previous
import os
import sys

for _p in ("/opt/trn_rl_repo", os.path.expanduser("~/.axon_site/_ro/trn_rl_repo")):
    if os.path.isdir(_p) and _p not in sys.path:
        sys.path.insert(0, _p)

import numpy as np
import ml_dtypes

import concourse.bass as bass
from concourse import bacc
import concourse.tile as tile
import concourse.mybir as mybir
from concourse.bass_utils import run_bass_kernel_spmd

# Problem shape (hardcoded per contract)
B, T, D, H, DK = 4, 2048, 1024, 16, 64
NCORES = 8

# Sharding: core = (batch b, head-group hg). Each core handles 8 heads of one
# batch over the full sequence, row-shards W_o, and the host sums the two
# partial outputs per batch (the "all-reduce" of the tensor-parallel scheme).
HC = H // 2       # 8 heads per core
DC = HC * DK      # 512 hidden dims per core

P = 128
NDT = D // P      # 8 din tiles
NHT = DC // P     # 4 dout tiles (= head pairs) for this core's heads
NKT = T // P      # 16 key-token tiles
NPAIR = HC // 2   # 4 head pairs (pair p <-> dout tile p)
QCH = 512         # free-dim chunk per q-chunk
NQC = T // QCH    # 4 q-chunks
NG = NKT // 2     # 8 kt-groups (2 kt each) per (chunk, pair) block

bf16 = mybir.dt.bfloat16
fp8 = mybir.dt.float8e4
f32 = mybir.dt.float32
FT = mybir.ActivationFunctionType
ADD = mybir.AluOpType.add
MUL = mybir.AluOpType.mult
DR = mybir.MatmulPerfMode.DoubleRow

_CACHE = {}

# Software-pipeline tuning: PE filler budget granted per exp-period (cycles at
# 2.4 GHz). The Act engine (exp stream) paces the kernel at ~2076 ns / period.
SLACK_CYC = 2100
BUDGET_CAP = 4 * SLACK_CYC
PROLOGUE_BUDGET = 8000


def build_kernel():
    nc = bacc.Bacc("TRN2", target_bir_lowering=False, debug=False, num_devices=1)

    # Per-core inputs, pre-tiled on the host so every DMA is one contiguous
    # transfer (HWDGE dispatch is ~625ns per DMA, so fewer/bigger is better):
    # Wq/Wk: [p, dt, a, m] = W[a*128+p, dt*128+m]; Wv: [p, a, m] = W[a*128+p, m]
    # Wo: [p, ch, a, m] = Wo_shard[a*128+p, ch*512+m]
    xT = nc.dram_tensor("xT", [D, T], bf16, kind="ExternalInput")  # x[b].T
    Wq = nc.dram_tensor("Wq", [P, NHT, NDT, P], bf16, kind="ExternalInput")
    Wk = nc.dram_tensor("Wk", [P, NHT, NDT, P], bf16, kind="ExternalInput")
    Wv = nc.dram_tensor("Wv", [P, NDT, DC], bf16, kind="ExternalInput")
    Wo = nc.dram_tensor("Wo", [P, 2, NHT, QCH], bf16, kind="ExternalInput")
    # bq pre-striped on host to [128, NHT] (col t = bias[t*128:(t+1)*128]).
    # The K bias is dropped entirely: softmax over keys is invariant to the
    # (q+bq)@bk term, so scores use (q+bq)@k with k = x@Wk (no bias).
    bqp = nc.dram_tensor("bqp", [P, NHT], f32, kind="ExternalInput")
    bv = nc.dram_tensor("bv", [1, DC], f32, kind="ExternalInput")
    bo = nc.dram_tensor("bo", [1, D], f32, kind="ExternalInput")  # pre-halved
    out = nc.dram_tensor("out", [T, D], mybir.dt.float16, kind="ExternalOutput")

    xTv = xT[:].rearrange("(a p) t -> p a t", p=P)  # [128, NDT, T] view

    with tile.TileContext(nc) as tc:
        with (
            tc.tile_pool(name="big", bufs=1) as big,
            tc.tile_pool(name="ptp", bufs=12) as ptp,
            tc.tile_pool(name="opk", bufs=8) as opkp,
            tc.tile_pool(name="rec", bufs=2) as recp,
            tc.tile_pool(name="res", bufs=4) as resp,
            tc.tile_pool(name="part", bufs=4) as partp,
            tc.tile_pool(name="sg", bufs=1, space="PSUM") as sgp,
            tc.tile_pool(name="acc", bufs=1, space="PSUM") as accp,
            tc.tile_pool(name="ops", bufs=2, space="PSUM") as opsp,
        ):
            # ---------------- static SBUF tiles + input DMA ----------------
            wk_w = big.tile([P, NHT, NDT, P], bf16, name="wk_w")
            wq_w = big.tile([P, NHT, NDT, P], bf16, name="wq_w")
            xt_sb = big.tile([P, NDT, T], bf16, name="xt_sb")
            bq_sb = big.tile([P, NHT], f32, name="bq_sb")
            bv_rep = big.tile([P, DC], f32, name="bv_rep")
            bo_rep = big.tile([P, D], f32, name="bo_rep")
            wv_ch = big.tile([P, NDT, DC], bf16, name="wv_ch")
            wo_ch = big.tile([P, 2, NHT, QCH], bf16, name="wo_ch")

            # fp8 Q/K for DoubleRow scores. Layout [p, i, t]: partition p<64 =
            # head-A dims, p>=64 = head-B dims; i is the DoubleRow k-tile dim
            # (tile 1 kept zero so contraction-64 runs at 0.5 cyc/row).
            kt_f8 = [big.tile([P, 2, T], fp8, name=f"ktf{p}") for p in range(NPAIR)]
            qt_f8 = [big.tile([P, 2, T], fp8, name=f"qtf{p}") for p in range(NPAIR)]
            # V with bias, ones column at [:, h, 64] for softmax denominators
            vp_sb = [big.tile([P, HC, DK + 1], bf16, name=f"vp{t}") for t in range(NKT)]
            # attention outputs, transposed: [din of pair, tt, token]
            ob_sb = [big.tile([P, NKT, P], bf16, name=f"ob{p}") for p in range(NPAIR)]

            # input loads, priority order, all on SP/HWDGE. The first
            # scores' critical path is Wk[pair0] -> x[chunk0] -> Wq[pair0],
            # so those go first as small DMAs.
            nc.sync.dma_start(wk_w[:, 0], Wk[:, 0])
            nc.sync.dma_start(xt_sb[:, :, 0:QCH], xTv[:, :, 0:QCH])
            nc.sync.dma_start(wq_w[:, 0], Wq[:, 0])
            nc.sync.dma_start(bq_sb[:], bqp[:])
            nc.sync.dma_start(wk_w[:, 1:NHT], Wk[:, 1:NHT])
            nc.sync.dma_start(xt_sb[:, :, QCH : 2 * QCH], xTv[:, :, QCH : 2 * QCH])
            nc.sync.dma_start(wq_w[:, 1:NHT], Wq[:, 1:NHT])
            nc.sync.dma_start(wv_ch[:], Wv[:])
            for q in range(2, 4):
                sl = slice(q * QCH, (q + 1) * QCH)
                nc.sync.dma_start(xt_sb[:, :, sl], xTv[:, :, sl])
            nc.sync.dma_start(bv_rep[:], bv[:].to_broadcast((P, DC)))
            nc.sync.dma_start(bo_rep[:], bo[:].to_broadcast((P, D)))
            nc.sync.dma_start(wo_ch[:], Wo[:])

            # memsets on Pool (otherwise idle): DoubleRow zero-tiles for pair 0
            # first (needed by the first scores), then denominators' ones
            # column, then the remaining pairs.
            nc.gpsimd.memset(kt_f8[0][:, 1, :], 0.0)
            nc.gpsimd.memset(qt_f8[0][:, 1, :], 0.0)
            for t in range(NKT):
                nc.gpsimd.memset(vp_sb[t][:, :, DK : DK + 1], 1.0)
            for p in range(1, NPAIR):
                nc.gpsimd.memset(kt_f8[p][:, 1, :], 0.0)
                nc.gpsimd.memset(qt_f8[p][:, 1, :], 0.0)

            # ---------------- work items (PE filler) ----------------
            def do_K(p, ch):
                sl = slice(ch * QCH, (ch + 1) * QCH)
                ps = opsp.tile([P, QCH], f32, tag="pps", name="pps")
                for di in range(NDT):
                    nc.tensor.matmul(
                        ps[:], wk_w[:, p, di, :], xt_sb[:, di, sl],
                        start=(di == 0), stop=(di == NDT - 1),
                    )
                nc.vector.tensor_copy(kt_f8[p][:, 0, sl], ps[:])

            def do_Q(c, p):
                sl = slice(c * QCH, (c + 1) * QCH)
                ps = opsp.tile([P, QCH], f32, tag="pps", name="pps")
                for di in range(NDT):
                    nc.tensor.matmul(
                        ps[:], wq_w[:, p, di, :], xt_sb[:, di, sl],
                        start=(di == 0), stop=(di == NDT - 1),
                    )
                nc.vector.tensor_tensor(
                    qt_f8[p][:, 0, sl], ps[:],
                    bq_sb[:, p : p + 1].to_broadcast((P, QCH)), ADD,
                )

            def do_V(p, tt):
                # one pair's 128 V columns for token tile tt
                ps = opsp.tile([P, QCH], f32, tag="pps", name="pps")
                csl = slice(p * P, (p + 1) * P)
                for di in range(NDT):
                    nc.tensor.matmul(
                        ps[:, 0:P], xt_sb[:, di, tt * P : (tt + 1) * P],
                        wv_ch[:, di, csl],
                        start=(di == 0), stop=(di == NDT - 1),
                    )
                nc.vector.tensor_tensor(
                    vp_sb[tt][:, 2 * p : 2 * p + 2, 0:DK],
                    ps[:, 0:P].rearrange("q (h d) -> q h d", d=DK),
                    bv_rep[:, csl].rearrange("q (h d) -> q h d", d=DK),
                    ADD,
                )

            out_v = out[:].rearrange("(tt p) d -> p tt d", p=P)
            part_tiles = {}

            def do_OP(ch, tg):
                # 2 token tiles -> one grouped result tile -> one DMA
                res = resp.tile([P, 2, QCH], mybir.dt.float16, tag="ores", name="ores")
                for k in range(2):
                    ttk = 2 * tg + k
                    ps = opsp.tile([P, QCH], f32, tag="pps", name="pps")
                    for p in range(NPAIR):
                        nc.tensor.matmul(
                            ps[:], ob_sb[p][:, ttk, :], wo_ch[:, ch, p, :],
                            start=(p == 0), stop=(p == NPAIR - 1),
                        )
                    nc.vector.tensor_tensor(
                        res[:, k, :], ps[:], bo_rep[:, ch * QCH : (ch + 1) * QCH], ADD
                    )
                nc.sync.dma_start(
                    out_v[:, 2 * tg : 2 * tg + 2, ch * QCH : (ch + 1) * QCH], res[:]
                )

            def do_OPP(ch, tg):
                # partial out-proj (pairs 0..2) + bias, staged to SBUF so the
                # final (pair-3) contribution is all that's left for the tail
                part = partp.tile([P, 2, QCH], bf16, tag="part", name="part")
                for k in range(2):
                    ttk = 2 * tg + k
                    ps = opsp.tile([P, QCH], f32, tag="pps", name="pps")
                    for p in range(NPAIR - 1):
                        nc.tensor.matmul(
                            ps[:], ob_sb[p][:, ttk, :], wo_ch[:, ch, p, :],
                            start=(p == 0), stop=(p == NPAIR - 2),
                        )
                    nc.vector.tensor_tensor(
                        part[:, k, :], ps[:], bo_rep[:, ch * QCH : (ch + 1) * QCH], ADD
                    )
                part_tiles[(ch, tg)] = part

            def do_OPF(ch, tg):
                part = part_tiles.pop((ch, tg))
                res = resp.tile([P, 2, QCH], mybir.dt.float16, tag="ores", name="ores")
                for k in range(2):
                    ttk = 2 * tg + k
                    ps = opsp.tile([P, QCH], f32, tag="pps", name="pps")
                    nc.tensor.matmul(
                        ps[:], ob_sb[NPAIR - 1][:, ttk, :], wo_ch[:, ch, NPAIR - 1, :],
                        start=True, stop=True,
                    )
                    nc.vector.tensor_tensor(res[:, k, :], ps[:], part[:, k, :], ADD)
                nc.sync.dma_start(
                    out_v[:, 2 * tg : 2 * tg + 2, ch * QCH : (ch + 1) * QCH], res[:]
                )

            COSTS = {"K": 4096, "Q": 4096, "V": 1024, "OP": 4096, "OPP": 3072,
                     "OPF": 1024}
            EMIT = {"K": do_K, "Q": do_Q, "V": do_V, "OP": do_OP, "OPP": do_OPP,
                    "OPF": do_OPF}

            state = {"budget": 0}
            emitted = set()
            queue = []  # ordered filler keys

            def emit_item(key):
                if key in emitted:
                    return
                emitted.add(key)
                EMIT[key[0]](*key[1:])
                state["budget"] -= COSTS[key[0]]

            def pump():
                while queue and state["budget"] > 0:
                    key = queue.pop(0)
                    emit_item(key)

            # filler queue: V per (pair, tt) so early blocks only need pair 0's
            # V; K chunks get pull-emitted exactly when scores need them.
            for p in range(NPAIR):
                for ch in range(4):
                    queue.append(("K", p, ch))
                for tt in range(NKT):
                    queue.append(("V", p, tt))

            # ---------------- phase 2 machinery ----------------
            blocks = [(c, p) for p in range(NPAIR) for c in range(NQC)]
            pt_tiles = {}     # (bi, g, head) -> pt AP
            av_pending = []   # (bi, qt) in emission order
            norm_cnt = [0] * NQC

            def emit_scores(bi, g):
                c, p = blocks[bi]
                qsl = slice(c * QCH, (c + 1) * QCH)
                for head, base, tag in ((0, 0, "sgA"), (1, 64, "sgB")):
                    sg = sgp.tile([P, 2, QCH], f32, tag=tag, name=tag)
                    for j in range(2):
                        kt = 2 * g + j
                        ksl = slice(kt * P, (kt + 1) * P)
                        nc.tensor.matmul(
                            sg[:, j, :],
                            kt_f8[p][base : base + DK, :, ksl],
                            qt_f8[p][base : base + DK, :, qsl],
                            start=True, stop=True,
                            perf_mode=DR,
                            tile_position=(base, 0),
                        )
                    pt = ptp.tile([P, 2, QCH], bf16, tag=f"pt{head}", name="pt")
                    nc.scalar.activation(pt[:], sg[:], FT.Exp, scale=0.125)
                    pt_tiles[(bi, g, head)] = pt
                state["budget"] -= 1024

            def emit_chain(bi, qt):
                # AV for one query tile: per head, a 16-kt accumulation chain
                # in an exclusive PSUM bank (one open group per 2KB zero
                # region), then normalize + transpose out.
                c, p = blocks[bi]
                if qt == 0:
                    for tt in range(NKT):
                        emit_item(("V", p, tt))
                qsl = slice(qt * P, (qt + 1) * P)
                opk = opkp.tile([P, P], bf16, tag="opk", name="opk")
                for head in (0, 1):
                    acc = accp.tile([P, QCH], f32, tag=f"acc{head}", name="acc")
                    h = 2 * p + head
                    for kt in range(NKT):
                        nc.tensor.matmul(
                            acc[:, 0 : DK + 1],
                            pt_tiles[(bi, kt // 2, head)][:, kt % 2, qsl],
                            vp_sb[kt][:, h, :],
                            start=(kt == 0),
                            stop=(kt == NKT - 1),
                        )
                    rec = recp.tile([P, 1], f32, tag=f"rec{head}", name="rec")
                    nc.vector.reciprocal(rec[:], acc[:, DK : DK + 1])
                    nc.vector.tensor_tensor(
                        opk[:, head * DK : (head + 1) * DK], acc[:, 0:DK],
                        rec[:].to_broadcast((P, DK)), MUL,
                    )
                nc.sync.dma_start_transpose(ob_sb[p][:, c * NQC + qt, :], opk[:])
                state["budget"] -= 2080
                if bi == len(blocks) - 1 and qt in (1, NQC - 1):
                    # last block: its chunk's final out-proj can start as soon
                    # as the needed token tiles are transposed
                    tg = 2 * c + (0 if qt == 1 else 1)
                    for ch in range(2):
                        emit_item(("OPP", ch, tg))
                        emit_item(("OPF", ch, tg))
                if qt == NQC - 1:
                    for g in range(NG):
                        for head in (0, 1):
                            del pt_tiles[(bi, g, head)]
                    norm_cnt[c] += 1
                    if c == NQC - 1:
                        # last chunk: staged partials once pairs 0..2 done,
                        # finals once pair 3 lands (keeps the tail short)
                        if norm_cnt[c] == NPAIR - 1:
                            for ch in range(2):
                                for tg in (2 * c, 2 * c + 1):
                                    queue.append(("OPP", ch, tg))
                        elif norm_cnt[c] == NPAIR:
                            for ch in range(2):
                                for tg in (2 * c, 2 * c + 1):
                                    queue.append(("OPF", ch, tg))
                    elif norm_cnt[c] == NPAIR:
                        for ch in range(2):
                            for tg in (2 * c, 2 * c + 1):
                                queue.append(("OP", ch, tg))

            def drain_av():
                cap = 1 if len(av_pending) <= 2 * NQC else 2
                n = 0
                while av_pending and n < cap:
                    bi, qt = av_pending[0]
                    if exp_done[0] < (bi + 1) * NG:
                        break
                    av_pending.pop(0)
                    emit_chain(bi, qt)
                    n += 1

            exp_done = [0]

            # ---------------- prologue + main loop ----------------
            emit_item(("K", 0, 0))
            emit_item(("Q", 0, 0))
            state["budget"] = PROLOGUE_BUDGET

            for bi, (c, p) in enumerate(blocks):
                emit_item(("Q", c, p))
                for g in range(NG):
                    emit_item(("K", p, g // 2))
                    emit_scores(bi, g)
                    exp_done[0] += 1
                    drain_av()
                    if g == NG - 2 and bi + 1 < len(blocks):
                        # pre-pull the next block's projections so its first
                        # scores are never gated on a just-emitted K/Q
                        cn, pn = blocks[bi + 1]
                        emit_item(("K", pn, 0))
                        emit_item(("Q", cn, pn))
                    state["budget"] += SLACK_CYC
                    pump()
                    if state["budget"] > BUDGET_CAP:
                        state["budget"] = BUDGET_CAP
                for qt in range(NQC):
                    av_pending.append((bi, qt))

            # tail: drain everything
            state["budget"] = 1 << 30
            while av_pending:
                bi, qt = av_pending.pop(0)
                emit_chain(bi, qt)
            pump()

    nc.compile()
    return nc


def _prep_inputs(x, Wq, bq, Wk, bk, Wv, bv, Wo, bo):
    """Shard + lay out inputs for the 8 cores (batch x head-group)."""
    x = np.asarray(x, dtype=np.float32)
    to_bf = lambda a: np.ascontiguousarray(a).astype(ml_dtypes.bfloat16)
    Wq, Wk, Wv, Wo = (np.asarray(w, np.float32) for w in (Wq, Wk, Wv, Wo))
    bq, bv, bo = (np.asarray(v, np.float32) for v in (bq, bv, bo))
    bo_half = np.ascontiguousarray((bo * 0.5).reshape(1, D))
    xTb = [to_bf(x[b].T) for b in range(B)]
    in_maps = []
    for core in range(NCORES):
        b, hg = core // 2, core % 2
        csl = slice(hg * DC, (hg + 1) * DC)

        def tile_qk(W):
            # [D, DC] -> [p, dt, a, m]
            return to_bf(
                W[:, csl].reshape(NDT, P, NHT, P).transpose(1, 2, 0, 3)
            )

        in_maps.append(
            {
                "xT": xTb[b],
                "Wq": tile_qk(Wq),
                "Wk": tile_qk(Wk),
                "Wv": to_bf(Wv[:, csl].reshape(NDT, P, DC).transpose(1, 0, 2)),
                "Wo": to_bf(
                    Wo[csl, :].reshape(NHT, P, 2, QCH).transpose(1, 2, 0, 3)
                ),
                "bqp": np.ascontiguousarray(bq[csl].reshape(NHT, P).T),
                "bv": np.ascontiguousarray(bv[csl].reshape(1, DC)),
                "bo": bo_half,
            }
        )
    return in_maps


def kernel(x, Wq, bq, Wk, bk, Wv, bv, Wo, bo):
    if "nc" not in _CACHE:
        _CACHE["nc"] = build_kernel()
    nc = _CACHE["nc"]
    in_maps = _prep_inputs(x, Wq, bq, Wk, bk, Wv, bv, Wo, bo)
    res = run_bass_kernel_spmd(nc, in_maps, list(range(NCORES)))
    out = np.empty((B, T, D), dtype=np.float32)
    for b in range(B):
        out[b] = res.results[2 * b]["out"].astype(np.float32) + res.results[
            2 * b + 1
        ]["out"].astype(np.float32)
    return out
